# revision 13
# baseline (speedup 1.0000x reference)
"""Trainium2 Bass kernel for a 2-layer shared-weight LSTM with residual.

Problem: x:[1024,200,128], W/U:[128,512], b:[512]; two stacked LSTM layers
sharing (W,U,b); layer 2 has a residual connection; seq_len ignored.

Sharding: data-parallel over batch: 1024 = 8 cores x 128 rows.
Orientation: features/gates on SBUF partitions, batch on the free axis.

Unit u computes L1 step u and L2 step u-1.  Structure vs the old
per-gate-ACT design:

* PSUM: one [128,1024] tile = 2 banks: bank A = [f i g o] of L1, bank B =
  [f i g o] of L2.  Gates reordered host-side from Keras (i,f,g,o) to
  (f,i,g,o); g columns pre-scaled x2 so ALL gates use sigmoid
  (tanh(zg) = 2*sig(2 zg) - 1).
* Bias pre-seeded into each bank by an fp32 identity-stationary matmul
  (rhs = bias pattern broadcast per gate block, start=True bank opener).
  This frees activations from the per-partition-bias constraint ->
  gates run as THREE large sigmoid ACTs instead of eight small ones:
    ACT_fig : cols   0:384  (f,i,g of L1) -- critical path
    ACT_r1  : cols 384:896  (o1, f2, i2, g2)
    ACT_o2  : cols 896:1024 (o2)
  plus tanh(c1) (critical) and tanh(c2) (slack).
* L2 recurrent input kept as h2raw + h1 split across two matmuls
  (residual add off the recurrence); h2raw(u-2) itself is materialized
  by a DVE mul DEFERRED to unit u (after tanh_c2(u-1)), feeding the
  U.h2raw early matmuls of unit u mid-burst.
* Pointwise ladders via scalar_tensor_tensor (both layers on DVE):
    u* = (g^-0.5)*i^ ; m* = f^*c ; c' = 2*u* + m* ; h = o^*tanh(c')
  GpSimd only does the y residual add (off-recurrence).
"""

import numpy as np
import ml_dtypes

import concourse.bass as bass
import concourse.tile as tile
from concourse import bacc, mybir
from concourse.bass_utils import run_bass_kernel_spmd

B, T, D = 1024, 200, 128
NCORES = 8
BL = B // NCORES  # 128 batch rows per core

F32 = mybir.dt.float32
BF16 = mybir.dt.bfloat16

# bank slice order (after host reorder): f, i, g, o
F, I, G, O = 0, 1, 2, 3


def _build(nc):
    x_d = nc.dram_tensor("x", [T, D, BL], BF16, kind="ExternalInput")
    w_d = nc.dram_tensor("w", [D, 4 * D], BF16, kind="ExternalInput")
    u_d = nc.dram_tensor("u", [D, 4 * D], BF16, kind="ExternalInput")
    bp_d = nc.dram_tensor("biaspat", [D, 4 * D], BF16, kind="ExternalInput")
    id_d = nc.dram_tensor("ident", [D, D], BF16, kind="ExternalInput")
    y_d = nc.dram_tensor("y", [T, D, BL], BF16, kind="ExternalOutput")

    SIG = mybir.ActivationFunctionType.Sigmoid
    TANH = mybir.ActivationFunctionType.Tanh
    MUL = mybir.AluOpType.mult
    ADD = mybir.AluOpType.add
    SUB = mybir.AluOpType.subtract

    def sA(k):
        return slice(k * BL, (k + 1) * BL)

    def sB(k):
        return slice(512 + k * BL, 512 + (k + 1) * BL)

    with tile.TileContext(nc) as tc:
        with (
            tc.tile_pool(name="singles", bufs=1) as singles,
            tc.tile_pool(name="psum", bufs=2, space="PSUM") as pspool,
            tc.tile_pool(name="gs", bufs=2) as gpool,
            tc.tile_pool(name="hb", bufs=4) as hpool,
            tc.tile_pool(name="hr", bufs=3) as hrpool,
            tc.tile_pool(name="xb", bufs=4) as xpool,
            tc.tile_pool(name="yt", bufs=3) as ypool,
            tc.tile_pool(name="tmp", bufs=3) as tpool,
        ):
            w_sb = singles.tile([D, 4 * D], BF16)
            u_sb = singles.tile([D, 4 * D], BF16)
            bp_sb = singles.tile([D, 4 * D], BF16)
            id_sb = singles.tile([D, D], BF16)
            nc.sync.dma_start(w_sb[:], w_d[:])
            nc.sync.dma_start(u_sb[:], u_d[:])
            nc.sync.dma_start(bp_sb[:], bp_d[:])
            nc.sync.dma_start(id_sb[:], id_d[:])

            # persistent cell state: cols 0:BL = c2, cols BL:2BL = c1 (bf16)
            c_both = singles.tile([D, 2 * BL], BF16)
            nc.vector.memset(c_both[:], 0.0)
            c2s = c_both[:, 0:BL]
            c1s = c_both[:, BL:2 * BL]

            def wk(k):
                return w_sb[:, k * D:(k + 1) * D]

            def uk(k):
                return u_sb[:, k * D:(k + 1) * D]

            _psn = [0]

            def new_ps():
                _psn[0] += 1
                return pspool.tile([D, 8 * BL], F32,
                                   tag=f"ps{_psn[0] % 2}", name="ps")

            _gsn = [0]

            def new_gs():
                _gsn[0] += 1
                return gpool.tile([D, 8 * BL], BF16,
                                  tag=f"gs{_gsn[0] % 2}", name="gs")

            def new_t(nm):
                return tpool.tile([D, BL], BF16, tag=nm, name=nm)

            xb = {}

            def load_x(t):
                if t < T:
                    xb[t] = xpool.tile([D, BL], BF16, tag="xb", name="xb")
                    nc.sync.dma_start(xb[t][:], x_d[t])

            load_x(0)
            load_x(1)

            h1 = {}     # h1[u]
            h2r = {}    # h2r[v] = h2raw(v), materialized in unit v+2
            gsd = {}    # gs tile of unit u (o2 slice read next unit)
            tc2d = {}   # tc2 tile of unit u (read next unit)

            # ---------------- unit 0: L1 step 0 only ----------------
            ps = new_ps()
            gs = new_gs()
            nc.tensor.matmul(ps[:, 0:512], id_sb[:], bp_sb[:, 0:512],
                             start=True, stop=False)
            for k in (F, I, G, O):
                nc.tensor.matmul(ps[:, sA(k)], wk(k), xb[0][:],
                                 start=False, stop=True)
            load_x(2)
            nc.scalar.activation(gs[:, 0:384], ps[:, 0:384], SIG)
            nc.scalar.activation(gs[:, 384:512], ps[:, 384:512], SIG)
            u1t = new_t("u1")
            tc1 = new_t("tc1")
            h1[0] = hpool.tile([D, BL], BF16, tag="hb", name="hb")
            nc.vector.scalar_tensor_tensor(
                u1t[:], gs[:, sA(G)], 0.5, gs[:, sA(I)], SUB, MUL)
            nc.vector.tensor_scalar_mul(c1s, u1t[:], 2.0)
            nc.scalar.activation(tc1[:], c1s, TANH)
            nc.vector.tensor_mul(h1[0][:], gs[:, sA(O)], tc1[:])

            # -------- units 1..T-1: L1 step u + L2 step u-1 --------
            # Pipelined emission: each unit emits its own LATES first, then
            # PREFETCH-emits unit u+1's bias/Wx/U.h1old so the PE stream is
            # [lates(u), Uh2raw(u), bias(u+1), Wx(u+1), Uh1old(u+1),
            #  lates(u+1), ...], pinned with sync=False order deps.
            prev_mm = None

            def mm(out, lhsT, rhs, start, stop):
                nonlocal prev_mm
                h = nc.tensor.matmul(out, lhsT, rhs, start=start, stop=stop)
                if prev_mm is not None:
                    tile.add_dep_helper(h.ins, prev_mm.ins, sync=False,
                                        reason="PE order pin")
                prev_mm = h
                return h

            # pre-allocate unit-1 psum/gates and emit its bias/Wx group
            ps_cur = new_ps()
            gs_cur = new_gs()
            mm(ps_cur[:, 0:512], id_sb[:], bp_sb[:, 0:512], True, False)
            mm(ps_cur[:, 512:1024], id_sb[:], bp_sb[:, 0:512], True, False)
            for k in (F, I, G, O):
                mm(ps_cur[:, sA(k)], wk(k), xb[1][:], False, False)

            for u in range(1, T):
                ps = ps_cur
                gs = gs_cur

                # ---- DVE-first: deferred h2raw(u-2) = o2^(u-1)*tc2(u-1)
                if u >= 2:
                    h2r[u - 2] = hrpool.tile([D, BL], BF16, tag="hr",
                                             name="hr")
                    nc.vector.tensor_mul(h2r[u - 2][:],
                                         gsd[u - 1][:, sB(O)],
                                         tc2d[u - 1][:])
                    ytt = ypool.tile([D, BL], BF16, tag="yt", name="yt")
                    nc.gpsimd.tensor_add(ytt[:], h2r[u - 2][:],
                                         h1[u - 2][:])
                    nc.sync.dma_start(y_d[u - 2], ytt[:])

                # ---- PE: LATES of unit u ----
                for k in (F, I, G, O):
                    mm(ps[:, sA(k)], uk(k), h1[u - 1][:], False, True)
                for k in (F, I, G, O):
                    mm(ps[:, sB(k)], wk(k), h1[u - 1][:], False,
                       u == 1)
                # ---- PE: U.h2raw(u) (ready mid-unit) ----
                if u >= 2:
                    for k in (F, I, G, O):
                        mm(ps[:, sB(k)], uk(k), h2r[u - 2][:], False, True)

                # ---- PE: prefetch unit u+1's bias/Wx/U.h1old ----
                if u + 1 < T:
                    ps_cur = new_ps()
                    gs_cur = new_gs()
                    mm(ps_cur[:, 0:512], id_sb[:], bp_sb[:, 0:512],
                       True, False)
                    mm(ps_cur[:, 512:1024], id_sb[:], bp_sb[:, 0:512],
                       True, False)
                    for k in (F, I, G, O):
                        mm(ps_cur[:, sA(k)], wk(k), xb[u + 1][:],
                           False, False)
                    for k in (F, I, G, O):
                        mm(ps_cur[:, sB(k)], uk(k), h1[u - 1][:],
                           False, False)

                load_x(u + 2)

                # ---- ScalarE: fig first; tanh_c1 emitted BEFORE r1 so
                # the critical fig wait cannot be merged upward with r1's
                # late U.h2raw dependency (different semaphore in between).
                nc.scalar.activation(gs[:, 0:384], ps[:, 0:384], SIG)

                # ---- DVE L1 ladder ----
                u1t = new_t("u1")
                m1t = new_t("m1")
                tc1 = new_t("tc1")
                u2t = new_t("u2")
                m2t = new_t("m2")
                tc2 = new_t("tc2")
                nc.vector.scalar_tensor_tensor(
                    u1t[:], gs[:, sA(G)], 0.5, gs[:, sA(I)], SUB, MUL)
                nc.vector.tensor_mul(m1t[:], gs[:, sA(F)], c1s)
                nc.vector.scalar_tensor_tensor(
                    c1s, u1t[:], 2.0, m1t[:], MUL, ADD)
                nc.scalar.activation(tc1[:], c1s, TANH)
                nc.scalar.activation(gs[:, 384:896], ps[:, 384:896], SIG)
                nc.vector.scalar_tensor_tensor(
                    u2t[:], gs[:, sB(G)], 0.5, gs[:, sB(I)], SUB, MUL)
                nc.vector.tensor_mul(m2t[:], gs[:, sB(F)], c2s)
                h1[u] = hpool.tile([D, BL], BF16, tag="hb", name="hb")
                nc.vector.tensor_mul(h1[u][:], gs[:, sA(O)], tc1[:])
                nc.vector.scalar_tensor_tensor(
                    c2s, u2t[:], 2.0, m2t[:], MUL, ADD)

                nc.scalar.activation(gs[:, 896:1024], ps[:, 896:1024], SIG)
                nc.scalar.activation(tc2[:], c2s, TANH)
                gsd[u] = gs
                tc2d[u] = tc2

                h1.pop(u - 3, None)
                xb.pop(u - 1, None)
                gsd.pop(u - 1, None)
                tc2d.pop(u - 1, None)
                h2r.pop(u - 3, None)

            # ---------------- tail: h2raw(T-2), y(T-2), then unit T ----
            h2r[T - 2] = hrpool.tile([D, BL], BF16, tag="hr", name="hr")
            nc.vector.tensor_mul(h2r[T - 2][:], gsd[T - 1][:, sB(O)],
                                 tc2d[T - 1][:])
            ytt = ypool.tile([D, BL], BF16, tag="yt", name="yt")
            nc.gpsimd.tensor_add(ytt[:], h2r[T - 2][:], h1[T - 2][:])
            nc.sync.dma_start(y_d[T - 2], ytt[:])

            ps = new_ps()
            gs = new_gs()
            mm(ps[:, 512:1024], id_sb[:], bp_sb[:, 0:512], True, False)
            for k in (F, I, G, O):
                mm(ps[:, sB(k)], uk(k), h1[T - 2][:], False, False)
            for k in (F, I, G, O):
                mm(ps[:, sB(k)], uk(k), h2r[T - 2][:], False, False)
            for k in (F, I, G, O):
                mm(ps[:, sB(k)], wk(k), h1[T - 1][:], False, True)
            nc.scalar.activation(gs[:, 512:1024], ps[:, 512:1024], SIG)
            u2t = new_t("u2")
            m2t = new_t("m2")
            tc2 = new_t("tc2")
            nc.vector.scalar_tensor_tensor(
                u2t[:], gs[:, sB(G)], 0.5, gs[:, sB(I)], SUB, MUL)
            nc.vector.tensor_mul(m2t[:], gs[:, sB(F)], c2s)
            nc.vector.scalar_tensor_tensor(
                c2s, u2t[:], 2.0, m2t[:], MUL, ADD)
            nc.scalar.activation(tc2[:], c2s, TANH)
            hr = new_t("hrT")
            nc.vector.tensor_mul(hr[:], gs[:, sB(O)], tc2[:])
            ylast = ypool.tile([D, BL], BF16, tag="yt", name="yt")
            nc.gpsimd.tensor_add(ylast[:], hr[:], h1[T - 1][:])
            nc.sync.dma_start(y_d[T - 1], ylast[:])

    nc.finalize()
    return nc


_CACHED = {}


def _get_nc():
    if "nc" not in _CACHED:
        nc = bacc.Bacc("TRN2", target_bir_lowering=False, debug=False,
                       num_devices=NCORES)
        _CACHED["nc"] = _build(nc)
    return _CACHED["nc"]


def kernel(x, W, U, b, seq_len):
    assert x.shape == (B, T, D)
    nc = _get_nc()

    bf = ml_dtypes.bfloat16
    # Keras gate order i,f,g,o -> kernel order f,i,g,o ; g columns x2
    perm = [1, 0, 2, 3]

    def reorder(M):
        M4 = np.asarray(M, dtype=np.float32).reshape(D, 4, D)[:, perm, :].copy()
        M4[:, 2, :] *= 2.0  # g columns x2 (all-sigmoid form)
        return np.ascontiguousarray(M4.reshape(D, 4 * D).astype(bf))

    Wc = reorder(W)
    Uc = reorder(U)
    b4 = np.asarray(b, dtype=np.float32).reshape(4, D)[perm, :].copy()
    b4[2, :] *= 2.0
    # bias pattern [D, 512] fp32: cols 128k..128k+127 = b4[k] broadcast
    bpat = np.repeat(b4[:, :, None], D, axis=2)          # [4, D, D]
    bpat = np.ascontiguousarray(
        bpat.transpose(1, 0, 2).reshape(D, 4 * D).astype(bf))
    ident = np.ascontiguousarray(np.eye(D, dtype=np.float32).astype(bf))

    in_maps = []
    for c in range(NCORES):
        xc = np.ascontiguousarray(
            np.asarray(x[c * BL:(c + 1) * BL], dtype=np.float32)
            .transpose(1, 2, 0).astype(bf))  # [T, D, BL] bf16
        in_maps.append({"x": xc, "w": Wc, "u": Uc, "biaspat": bpat,
                        "ident": ident})

    res = run_bass_kernel_spmd(nc, in_maps, core_ids=list(range(NCORES)))

    y = np.empty((B, T, D), dtype=np.float32)
    for c in range(NCORES):
        y[c * BL:(c + 1) * BL] = (
            res.results[c]["y"].astype(np.float32).transpose(2, 0, 1))
    return y


# revision 14
# speedup vs baseline: 1.1589x; 1.1589x over previous
"""Trainium2 Bass kernel for a 2-layer shared-weight LSTM with residual.

Problem: x:[1024,200,128], W/U:[128,512], b:[512]; two stacked LSTM layers
sharing (W,U,b); layer 2 has a residual connection; seq_len is ignored by the
reference (full T steps).

Sharding: data-parallel over batch: 1024 = 8 cores x 128 rows.

Device layout ("orientation B"): features/gates on SBUF partitions, batch on
the free axis.  Host pre-transposes x to [T, D, B_local] (bf16) so each
timestep tile is [D=128 partitions, B=128 free] and DMAs straight in.

Fused-unit schedule: unit u (u=0..T) computes layer-2 step u-1 and layer-1
step u together.  For each gate chunk k the PSUM tile holds
    cols 0:128   = z2(u-1) = W_k h1(u-1) + U_k h2raw(u-2) + U_k h1(u-2) + b_k
    cols 128:256 = z1(u)   = W_k x(u)    + U_k h1(u-1)                  + b_k
The layer-2 recurrent input h2n = h2raw + h1 is split across two matmuls so
the residual add is off the recurrence cycle entirely (it only feeds the y
output DMA, on GpSimd).  Matmuls whose inputs are a unit old (U_k h1(u-2),
W_k x(u)) are issued early so only three N=128 matmuls sit between h-ready
and the first gate activation.  Gate activations are merged [128,256]
ScalarE ops (bias fused; per-partition because gates live on partitions);
the c/h pointwise tail is split into L1/L2 halves to shorten the serial
recurrence.  Matmuls run in bf16 (fp32 runs 2-pass LOW_HIGH at half speed); the whole
pointwise path including the c state is bf16 (DVE 2x mode; measured max
rel err 9.7e-3 on the full sequence).
"""

import numpy as np
import ml_dtypes

import concourse.bass as bass
import concourse.tile as tile
from concourse import bacc, mybir
from concourse.bass_utils import run_bass_kernel_spmd

B, T, D = 1024, 200, 128
NCORES = 8
BL = B // NCORES  # 128 batch rows per core

F32 = mybir.dt.float32
import os
BF16 = mybir.dt.float32 if os.environ.get("K_FP32") else mybir.dt.bfloat16

# gate order in W/U/b: i, f, g, o  (Keras LSTMCell)
GI, GF, GG, GO = 0, 1, 2, 3
CHUNKS = (GF, GI, GG, GO)  # f first: the c-path needs sig(f) earliest


def _build(nc):
    x_d = nc.dram_tensor("x", [T, D, BL], BF16, kind="ExternalInput")
    w_d = nc.dram_tensor("w", [D, 4 * D], BF16, kind="ExternalInput")
    u_d = nc.dram_tensor("u", [D, 4 * D], BF16, kind="ExternalInput")
    b_d = nc.dram_tensor("bias", [D, 4], F32, kind="ExternalInput")
    y_d = nc.dram_tensor("y", [T, D, BL], BF16, kind="ExternalOutput")

    SIG = mybir.ActivationFunctionType.Sigmoid
    TANH = mybir.ActivationFunctionType.Tanh

    L2 = slice(0, BL)           # layer-2 half (cols 0:128)
    L1 = slice(BL, 2 * BL)      # layer-1 half (cols 128:256)

    with tile.TileContext(nc) as tc:
        with (
            tc.tile_pool(name="singles", bufs=1) as singles,
            tc.tile_pool(name="hbuf", bufs=6) as hpool,
            tc.tile_pool(name="psum", bufs=2, space="PSUM") as pspool,
            tc.tile_pool(name="gates", bufs=3) as gpool,
            tc.tile_pool(name="yst", bufs=4) as ypool,
        ):
            w_sb = singles.tile([D, 4 * D], BF16)
            u_sb = singles.tile([D, 4 * D], BF16)
            b_sb = singles.tile([D, 4], F32)
            nc.sync.dma_start(w_sb[:], w_d[:])
            nc.sync.dma_start(u_sb[:], u_d[:])
            nc.sync.dma_start(b_sb[:], b_d[:])

            # persistent cell state: cols 0:128 = c2, cols 128:256 = c1
            # bf16 keeps the whole c'-path in the DVE 2x mode (measured
            # max rel err 9.7e-3 vs 8.6e-3 with fp32 c)
            c_both = singles.tile([D, 2 * BL], BF16)
            nc.vector.memset(c_both[:], 0.0)

            def wk(k):
                return w_sb[:, k * D:(k + 1) * D]

            def uk(k):
                return u_sb[:, k * D:(k + 1) * D]

            def bk(k):
                return b_sb[:, k:k + 1]

            # ring: hb[u] = [h2raw(u-1) | h1(u) | x(u+1)]   (bf16)
            def new_hb():
                return hpool.tile([D, 3 * BL], BF16, tag="hbuf", name="hbuf")

            def new_ps():
                return {k: pspool.tile([D, 2 * BL], F32, tag=f"ps{k}",
                                       name=f"ps{k}") for k in range(4)}

            def new_gates(names):
                return {n: gpool.tile([D, 2 * BL], BF16, tag=n, name=n)
                        for n in names}

            # hb_pre carries x(0) for unit 0
            hb_pre = new_hb()
            nc.sync.dma_start(hb_pre[:, 2 * BL:3 * BL], x_d[0])

            # ---------------- unit 0: layer-1 step 0 only ----------------
            # z1(0) = W x(0) + b ;  c1(0) = sig(i)*tanh(g) ; h1(0)=sig(o)*tanh(c1)
            hb0 = new_hb()
            nc.sync.dma_start(hb0[:, 2 * BL:3 * BL], x_d[1])
            ps = new_ps()
            for k in CHUNKS:
                nc.tensor.matmul(ps[k][:, L1], wk(k),
                                 hb_pre[:, 2 * BL:3 * BL],
                                 start=True, stop=True)
            g = new_gates(["gf", "gi", "gg", "go", "tc1", "tc2"])
            nc.scalar.activation(g["gi"][:, L1], ps[GI][:, L1], SIG, bias=bk(GI))
            nc.scalar.activation(g["gg"][:, L1], ps[GG][:, L1], TANH, bias=bk(GG))
            nc.scalar.activation(g["go"][:, L1], ps[GO][:, L1], SIG, bias=bk(GO))
            nc.vector.tensor_mul(c_both[:, L1], g["gi"][:, L1], g["gg"][:, L1])
            nc.scalar.activation(g["tc1"][:, L1], c_both[:, L1], TANH)
            nc.vector.tensor_mul(hb0[:, L1], g["go"][:, L1], g["tc1"][:, L1])

            hb = {-1: hb_pre, 0: hb0}
            prev_u2f = None

            # -------- units 1..T-1: fused L2(u-1) + L1(u) --------
            for u in range(1, T):
                hb_u = new_hb()
                hb[u] = hb_u
                if u + 1 < T:
                    nc.sync.dma_start(hb_u[:, 2 * BL:3 * BL], x_d[u + 1])

                ps = new_ps()
                # PSUM bank opener must be the FIRST matmul per bank
                # (start=True resets the bank's has_written).  Inputs that are
                # a unit old run EARLY (before h1(u-1) exists): the opener
                # U_k h1(u-2) and the W_k x(u) half.  Late (after h1/h2raw):
                # W_k h1(u-1), U_k h1(u-1), U_k h2raw(u-2) -- only three short
                # matmuls sit between h-ready and the first gate activation.
                g = new_gates(["gf", "gi", "gg", "go", "tc1", "tc2"])
                m = {n: gpool.tile([D, 2 * BL], BF16, tag=n, name=n)
                     for n in ("m1a", "m1b", "m2a", "m2b")}
                FUNC = {GF: SIG, GI: SIG, GG: TANH, GO: SIG}
                NM = {GF: "gf", GI: "gi", GG: "gg", GO: "go"}

                if u >= 2:
                    for k in CHUNKS:
                        mo = nc.tensor.matmul(ps[k][:, L2], uk(k),
                                              hb[u - 2][:, BL:2 * BL],
                                              start=True, stop=False)
                        mx = nc.tensor.matmul(ps[k][:, L1], wk(k),
                                              hb[u - 1][:, 2 * BL:3 * BL],
                                              start=False, stop=False)
                        if prev_u2f is not None:
                            # keep next-unit early matmuls out of the
                            # critical f-chunk window of THIS unit
                            for mm in (mo, mx):
                                tile.add_dep_helper(
                                    mm.ins, prev_u2f.ins, sync=False,
                                    reason="early after prev U_f(h2raw)")
                    mm_w = {}
                    mm_u2 = {}
                    # f z1-late FIRST: sig(f)-L1 gates the c1 path and now
                    # waits on a single N=128 matmul instead of three
                    mm_u1f = nc.tensor.matmul(ps[GF][:, L1], uk(GF),
                                              hb[u - 1][:, BL:2 * BL],
                                              start=False, stop=True)
                    nc.scalar.activation(g["gf"][:, L1], ps[GF][:, L1],
                                         SIG, bias=bk(GF))
                    for k in (GI, GG):
                        mm_w[k] = nc.tensor.matmul(
                            ps[k][:, L2], wk(k), hb[u - 1][:, BL:2 * BL],
                            start=False, stop=False)
                        nc.tensor.matmul(ps[k][:, L1], uk(k),
                                         hb[u - 1][:, BL:2 * BL],
                                         start=False, stop=True)
                        mm_u2[k] = nc.tensor.matmul(
                            ps[k][:, L2], uk(k), hb[u - 1][:, 0:BL],
                            start=False, stop=True)
                        nc.scalar.activation(g[NM[k]][:], ps[k][:],
                                             FUNC[k], bias=bk(k))
                    # f z2-lates + deferred sig(f)-L2 (feeds m1b, slack path)
                    mm_w[GF] = nc.tensor.matmul(
                        ps[GF][:, L2], wk(GF), hb[u - 1][:, BL:2 * BL],
                        start=False, stop=False)
                    mm_u2[GF] = nc.tensor.matmul(
                        ps[GF][:, L2], uk(GF), hb[u - 1][:, 0:BL],
                        start=False, stop=True)
                    nc.scalar.activation(g["gf"][:, L2], ps[GF][:, L2],
                                         SIG, bias=bk(GF))
                    # o lates last (sig(o) only feeds the h-muls after tanh)
                    mm_w[GO] = nc.tensor.matmul(
                        ps[GO][:, L2], wk(GO), hb[u - 1][:, BL:2 * BL],
                        start=False, stop=False)
                    nc.tensor.matmul(ps[GO][:, L1], uk(GO),
                                     hb[u - 1][:, BL:2 * BL],
                                     start=False, stop=True)
                    mm_u2[GO] = nc.tensor.matmul(
                        ps[GO][:, L2], uk(GO), hb[u - 1][:, 0:BL],
                        start=False, stop=True)
                    nc.scalar.activation(g["go"][:], ps[GO][:],
                                         FUNC[GO], bias=bk(GO))
                    # keep the PE late order: f-z1, i, g, f-z2, o
                    for a, b in ((mm_w[GI], mm_u1f), (mm_w[GG], mm_u2[GI]),
                                 (mm_w[GF], mm_u2[GG]), (mm_w[GO], mm_u2[GF])):
                        tile.add_dep_helper(a.ins, b.ins, sync=False,
                                            reason="chunk ladder order")
                    prev_u2f = mm_u2[GG]
                else:
                    for k in CHUNKS:
                        nc.tensor.matmul(ps[k][:, 0:2 * BL], wk(k),
                                         hb[u - 1][:, BL:3 * BL],
                                         start=True, stop=False)
                        nc.tensor.matmul(ps[k][:, L1], uk(k),
                                         hb[u - 1][:, BL:2 * BL],
                                         start=False, stop=True)
                        nc.scalar.activation(g[NM[k]][:], ps[k][:],
                                             FUNC[k], bias=bk(k))

                # L1 tail (critical recurrence): c1' -> tanh -> h1(u)
                nc.vector.tensor_mul(m["m1a"][:, L1], g["gf"][:, L1],
                                     c_both[:, L1])
                nc.vector.tensor_mul(m["m2a"][:, L1], g["gi"][:, L1],
                                     g["gg"][:, L1])
                nc.vector.tensor_add(c_both[:, L1], m["m1a"][:, L1],
                                     m["m2a"][:, L1])
                nc.scalar.activation(g["tc1"][:, L1], c_both[:, L1], TANH)
                # L2 tail: c2' -> tanh -> h2raw(u-1)
                nc.vector.tensor_mul(m["m1b"][:, L2], g["gf"][:, L2],
                                     c_both[:, L2])
                nc.vector.tensor_mul(m["m2b"][:, L2], g["gi"][:, L2],
                                     g["gg"][:, L2])
                nc.vector.tensor_mul(hb_u[:, L1], g["go"][:, L1],
                                     g["tc1"][:, L1])          # h1(u)
                nc.vector.tensor_add(c_both[:, L2], m["m1b"][:, L2],
                                     m["m2b"][:, L2])
                nc.scalar.activation(g["tc2"][:, L2], c_both[:, L2], TANH)
                nc.vector.tensor_mul(hb_u[:, L2], g["go"][:, L2],
                                     g["tc2"][:, L2])          # h2raw(u-1)

                # residual -> y(u-1): off the recurrence, on GpSimd
                yt = ypool.tile([D, BL], BF16, tag="yst", name="yst")
                nc.gpsimd.tensor_add(yt[:], hb_u[:, 0:BL],
                                     hb[u - 1][:, BL:2 * BL])
                nc.sync.dma_start(y_d[u - 1], yt[:])

                hb.pop(u - 3, None)

            # ---------------- unit T: layer-2 step T-1 only ----------------
            u = T
            ps = new_ps()
            for k in CHUNKS:
                nc.tensor.matmul(ps[k][:, L2], uk(k), hb[u - 2][:, BL:2 * BL],
                                 start=True, stop=False)
                nc.tensor.matmul(ps[k][:, L2], wk(k), hb[u - 1][:, BL:2 * BL],
                                 start=False, stop=False)
                nc.tensor.matmul(ps[k][:, L2], uk(k), hb[u - 1][:, 0:BL],
                                 start=False, stop=True)
            g = new_gates(["gf", "gi", "gg", "go", "tc2"])
            m = {n: gpool.tile([D, 2 * BL], BF16, tag=n, name=n)
                 for n in ("m1b", "m2b")}
            nc.scalar.activation(g["gf"][:, L2], ps[GF][:, L2], SIG, bias=bk(GF))
            nc.scalar.activation(g["gi"][:, L2], ps[GI][:, L2], SIG, bias=bk(GI))
            nc.scalar.activation(g["gg"][:, L2], ps[GG][:, L2], TANH, bias=bk(GG))
            nc.scalar.activation(g["go"][:, L2], ps[GO][:, L2], SIG, bias=bk(GO))
            nc.vector.tensor_mul(m["m1b"][:, L2], g["gf"][:, L2], c_both[:, L2])
            nc.vector.tensor_mul(m["m2b"][:, L2], g["gi"][:, L2], g["gg"][:, L2])
            nc.vector.tensor_add(c_both[:, L2], m["m1b"][:, L2], m["m2b"][:, L2])
            nc.scalar.activation(g["tc2"][:, L2], c_both[:, L2], TANH)
            hraw = ypool.tile([D, BL], BF16, tag="yst", name="hraw")
            nc.vector.tensor_mul(hraw[:], g["go"][:, L2], g["tc2"][:, L2])
            yt = ypool.tile([D, BL], BF16, tag="yst", name="yst")
            nc.gpsimd.tensor_add(yt[:], hraw[:], hb[u - 1][:, BL:2 * BL])
            nc.sync.dma_start(y_d[T - 1], yt[:])

    nc.finalize()
    return nc


_CACHED = {}


def _get_nc():
    if "nc" not in _CACHED:
        nc = bacc.Bacc("TRN2", target_bir_lowering=False, debug=False,
                       num_devices=NCORES)
        _CACHED["nc"] = _build(nc)
    return _CACHED["nc"]


def kernel(x, W, U, b, seq_len):
    assert x.shape == (B, T, D)
    nc = _get_nc()

    import os
    bf = np.float32 if os.environ.get("K_FP32") else ml_dtypes.bfloat16
    Wc = np.ascontiguousarray(np.asarray(W, dtype=np.float32).astype(bf))
    Uc = np.ascontiguousarray(np.asarray(U, dtype=np.float32).astype(bf))
    bc = np.ascontiguousarray(
        np.asarray(b, dtype=np.float32).reshape(4, D).T)  # [D, 4]

    in_maps = []
    for c in range(NCORES):
        xc = np.ascontiguousarray(
            np.asarray(x[c * BL:(c + 1) * BL], dtype=np.float32)
            .transpose(1, 2, 0).astype(bf))  # [T, D, BL] bf16
        in_maps.append({"x": xc, "w": Wc, "u": Uc, "bias": bc})

    res = run_bass_kernel_spmd(nc, in_maps, core_ids=list(range(NCORES)))

    y = np.empty((B, T, D), dtype=np.float32)
    for c in range(NCORES):
        # y_T [T, D, BL] bf16 -> [BL, T, D] fp32
        y[c * BL:(c + 1) * BL] = (
            res.results[c]["y"].astype(np.float32).transpose(2, 0, 1))
    return y

